# revision 7
# baseline (speedup 1.0000x reference)
"""CP-gate layer kernel for Trainium2 (8 NeuronCores, batch-parallel).

The reference materializes the dense 2^n x 2^n CP gate, but that matrix is
diagonal: diag entry is e^{-i*phase} on basis states where both the control
(bit 11, MSB) and target (bit 10) bits are 1, else 1.  With MSB-first
ordering those states are exactly the contiguous index range [3072, 4096).
So U @ psi is: identity on k < 3072, and a fixed complex rotation of the
tail quarter.  The batch of 64 state vectors is sharded across 8 cores
(8 states/core): each core DMA-copies the untouched 3/4 DRAM->DRAM and
rotates its tail quarter on the vector engine.

Raw manually-synced bacc (no TileContext), structured as one latency chain:
  SP:   tails DRAM->SBUF load (HWDGE), then the body DRAM->DRAM copy
  DVE:  3-op rotate chain (same-engine program order, no internal sems)
  Pool: kv_writeback store prepared at t~0.7us, triggered on the dve sem
All barrier EventSemaphores are stripped post-finalize; the per-engine
drains are kept (sync cleared).  Every cross-engine dependency flows
through the ld/dve/prep semaphores, and the end drains gate kernel
completion on DGE-queue quiescence directly, so the exit path never pays
the ~900ns DMA->semaphore propagation (the store's completion sem still
fires — neuronxcc requires DGE sync info — but nothing waits on it).
Sems are zeroed by the NRT preamble before any engine starts, so the
monotonic waits need no start barrier.

Critical chain (cost model): load 25+625(HWDGE)+650(DGE)+182(xfer)
+900(sem) = 2382 -> DVE 94+127+127+60(SBUF ack)+28(sem) -> trigger
+8+1+13(store xfer) -> +900 (store completion sem track) = 3747 ns.
"""

import numpy as np

N_CORES = 8
BATCH = 64
DIM = 4096
B_PER = BATCH // N_CORES          # 8 states per core
SPLIT = 3072                      # k >= SPLIT picks up the phase
TAIL = DIM - SPLIT                # 1024
NPART = 128                       # tail tile partitions: (b, km) = 8*16
HK = 64                           # tail tile cols per half: re 0:64, im 64:128
PHASE = np.pi / 4.0
C = float(np.cos(PHASE))          # cos == sin for pi/4

_cached_nc = None


def _build_nc():
    import concourse.bacc as bacc
    import concourse.bass as bass
    import concourse.mybir as mybir

    f32 = mybir.dt.float32
    i32 = mybir.dt.int32
    nc = bacc.Bacc("TRN2", target_bir_lowering=False, debug=False, num_devices=N_CORES)
    body = nc.declare_dram_parameter("body", [2, B_PER, SPLIT], f32, isOutput=False)
    tails = nc.declare_dram_parameter("tails", [NPART, 2 * HK], f32, isOutput=False)
    obody = nc.declare_dram_parameter("out_body", [2, B_PER, SPLIT], f32, isOutput=True)
    otail = nc.declare_dram_parameter("out_tail", [NPART, 2 * HK], f32, isOutput=True)

    with (
        nc.sbuf_tensor([NPART, 2 * HK], f32) as t,
        nc.sbuf_tensor([NPART, 2 * HK], f32) as s,
        nc.sbuf_tensor([NPART, 2 * HK], f32) as r,
        nc.Block() as block,
        nc.semaphore("ld") as ld,
        nc.semaphore("dve") as dve,
        nc.semaphore("cp") as cp,
        nc.semaphore("st") as st,
        nc.semaphore("prep") as prep,
    ):

        @block.sync
        def _(sp: bass.BassEngine):
            # Tail load first (critical path), then the independent body
            # copy.  Both HWDGE; the body's descriptor gen + DGE delay puts
            # its transfer at ~1.95-2.5us, between the tail load's transfer
            # (~1.3-1.5us) and the store trigger (~2.8us), so the three
            # never contend for the DMA engines.  The body copy carries no
            # completion sem: nothing consumes it, and end-of-kernel DMA
            # quiescence is owned by the engines' end drains, which observe
            # queue state directly instead of paying the ~900ns DMA->sem
            # propagation.  (The body copy still carries cp — neuronxcc
            # requires sync info on every DGE DMA — but nothing waits on it.)
            sp.dma_start(out=t[:], in_=tails[:]).then_inc(ld, 16)
            sp.dma_start(out=obody[:, :, :], in_=body[:, :, :]).then_inc(cp, 16)

        @block.gpsimd
        def _(g: bass.BassEngine):
            # Zero ctx index: reuse the preamble's const-f32-0.0 [128,1] SBUF
            # tensor (all-zero bytes) bitcast to int32; Pool program order
            # (memsets precede this) makes the read safe barrier-free.
            idx0 = nc.const_aps.aps[(f32, 0.0)].bitcast(i32)
            out4 = otail[:].rearrange("p (o n) -> p o n", o=1).unsqueeze(0)
            in4 = r[:].rearrange("p (a n) -> p a n", a=1).unsqueeze(2)
            g.kv_writeback(
                out_ap=out4, in_ap=in4, ctx_idxs_ap=idx0,
                prepare_only=True, sem=st, queue_num=0,
            ).then_inc(prep, 1)
            # Wait order matters: Bacc fuses one wait onto the trigger.
            # This order lands the critical dve wait ON the trigger ISA op
            # and leaves the early-satisfied prep wait standalone.  The
            # store's completion (st) is not waited on: Pool's end drain
            # covers the triggered queue.
            g.wait_ge(dve, 1)
            g.wait_ge(prep, 1)
            g.trigger_dma(count=1, queue_num=0)

        @block.vector
        def _(v: bass.BassEngine):
            v.wait_ge(ld, 16)
            # s_im = fl(C*im); then out_re = fl(C*re)+s_im, out_im = s_im-fl(C*re)
            # via scalar_tensor_tensor — same rounding as the reference.
            # No sems between the three ops: the DVE engine executes its
            # queue in program order, so the RAW on s is safe; only the
            # last op signals (the trigger needs all of r, and out_re's
            # columns are complete before out_im's by engine order).
            v.tensor_scalar_mul(s[:, HK : 2 * HK], t[:, HK : 2 * HK], C)
            v.scalar_tensor_tensor(
                out=r[:, 0:HK], in0=t[:, 0:HK], scalar=C, in1=s[:, HK : 2 * HK],
                op0=mybir.AluOpType.mult, op1=mybir.AluOpType.add,
            )
            v.scalar_tensor_tensor(
                out=r[:, HK : 2 * HK], in0=t[:, 0:HK], scalar=-C, in1=s[:, HK : 2 * HK],
                op0=mybir.AluOpType.mult, op1=mybir.AluOpType.add,
            ).then_inc(dve, 1)

    fn = nc.m.functions[0]

    # Strip the barrier event semaphores (start and end) but KEEP the drains,
    # with their barrier-sem waits/incs cleared.  All real ordering flows
    # through ld/dve/prep; the end drains are what gates kernel completion on
    # the in-flight DMAs (body copy + triggered store) — they observe DGE
    # queue quiescence directly, with no semaphore propagation on the exit
    # path.
    def _names(i):
        si = getattr(i, "sync_info", None)
        ow = (getattr(si, "on_wait", None) or []) + (getattr(si, "on_update", None) or [])
        return [getattr(w, "ant_name", "") or "" for w in ow]

    for b in fn.blocks:
        for i in list(b.instructions):
            if isinstance(i, mybir.InstDrain):
                i.sync_info = None
            elif isinstance(i, mybir.InstEventSemaphore) and any(
                n.startswith("barrier") for n in _names(i)
            ):
                b.instructions.remove(i)

    # Hoist both SP DMAs to the very top of block 0 so the tail load
    # dispatches at t~0 instead of after SP's block branch.
    SP = mybir.EngineType.SP
    hoisted = []
    for b in fn.blocks:
        for i in list(b.instructions):
            if isinstance(i, mybir.InstDMACopy) and i.engine == SP:
                hoisted.append(i)
                b.instructions.remove(i)
    assert len(hoisted) == 2, hoisted
    main = fn.blocks[0]
    pos = 1 if main.instructions and isinstance(main.instructions[0], mybir.InstCall) else 0
    main.instructions[pos:pos] = hoisted

    nc.finalize()
    return nc


def _get_nc():
    global _cached_nc
    if _cached_nc is None:
        _cached_nc = _build_nc()
    return _cached_nc


def kernel(psi_re=None, psi_im=None, U_re=None, U_im=None, _trace=False, **_ignored):
    from concourse.bass_utils import run_bass_kernel_spmd

    psi_re = np.asarray(psi_re, dtype=np.float32).reshape(BATCH, DIM)
    psi_im = np.asarray(psi_im, dtype=np.float32).reshape(BATCH, DIM)

    nc = _get_nc()
    in_maps = []
    for i in range(N_CORES):
        re = psi_re[i * B_PER : (i + 1) * B_PER]
        im = psi_im[i * B_PER : (i + 1) * B_PER]
        body = np.ascontiguousarray(np.stack([re[:, :SPLIT], im[:, :SPLIT]]))
        tails = np.concatenate(
            [re[:, SPLIT:].reshape(NPART, HK), im[:, SPLIT:].reshape(NPART, HK)],
            axis=1,
        )
        in_maps.append({"body": body, "tails": np.ascontiguousarray(tails)})

    if _trace:
        res = run_bass_kernel_spmd(nc, in_maps, list(range(N_CORES)), trace=True)
    else:
        res = run_bass_kernel_spmd(nc, in_maps, list(range(N_CORES)))

    out = np.empty((2, BATCH, DIM, 1), dtype=np.float32)
    for i in range(N_CORES):
        ob = res.results[i]["out_body"]            # (2, B_PER, SPLIT)
        ot = res.results[i]["out_tail"]            # (NPART, 2*HK)
        sl = slice(i * B_PER, (i + 1) * B_PER)
        out[0, sl, :SPLIT, 0] = ob[0]
        out[1, sl, :SPLIT, 0] = ob[1]
        out[0, sl, SPLIT:, 0] = ot[:, :HK].reshape(B_PER, TAIL)
        out[1, sl, SPLIT:, 0] = ot[:, HK:].reshape(B_PER, TAIL)
    if _trace:
        kernel.last_results = res
    return out
